# revision 27
# baseline (speedup 1.0000x reference)
"""MoE router kernel for Trainium2 (8 NeuronCores, data-parallel over tokens).

Computes, for x (N=16384, H=2048) f32, W (E=64, H) f32, b (E,) f32:
  router_logits  (N, E) f32   = x @ W.T + b
  router_weights (N, 2) f32   = renormalized top-2 softmax probs
  selected_idx   (N, 2) int32 = top-2 expert indices
  expert_mask    (E, 2, N) int32 = one-hot(selected_idx) transposed (2,1,0)

Sharding: token dim N split evenly across 8 cores; W/b replicated. Each
core's x shard is pre-arranged on the host into per-group slabs
[NG, 128, HK, GROUP] (hidden chunk on the partition axis) so the gate
matmul contracts over H directly from DMA-friendly 32KB-contiguous rows.
"""

import os
import sys

import numpy as np

if not os.path.isdir(os.path.join(os.path.dirname(os.path.abspath(__file__)), "concourse")):
    for _p in ("/opt/trn_rl_repo",):
        if os.path.isdir(_p) and _p not in sys.path:
            sys.path.insert(0, _p)

import concourse.bass as bass  # noqa: E402,F401
import concourse.bacc as bacc  # noqa: E402
import concourse.mybir as mybir  # noqa: E402
import concourse.tile as tile  # noqa: E402
from concourse.masks import make_identity  # noqa: E402

F32 = mybir.dt.float32
F32R = mybir.dt.float32r
I32 = mybir.dt.int32
U32 = mybir.dt.uint32

N = 16384
H = 2048
E = 64
TOP_K = 2
NCORES = 8
N_LOC = N // NCORES      # 2048 tokens per core
GROUP = 1024             # tokens per compute group
NG = N_LOC // GROUP      # 2 groups
NT = GROUP // 128        # 8 token sub-tiles per group
SUB = GROUP // 512       # 512-token subgroups (mask granularity)
HK = H // 128            # 16 contraction chunks
KH = HK // 4             # k-chunks per DMA quarter-slab
NQ = HK // KH            # DMA slabs per group
QROWS = N_LOC // 128     # 16 (for weights/indices packing)

MM_F32R = False
TR_F32R = False


def build_nc(mm_f32r=MM_F32R, tr_f32r=TR_F32R):
    nc = bacc.Bacc("TRN2", target_bir_lowering=False)

    xg_d = nc.dram_tensor("xg", [NG, 128, HK, GROUP], F32, kind="ExternalInput")
    w_d = nc.dram_tensor("W", [E, H], F32, kind="ExternalInput")
    b_d = nc.dram_tensor("b", [E], F32, kind="ExternalInput")
    logits_d = nc.dram_tensor("router_logits", [N_LOC, E], F32, kind="ExternalOutput")
    rw_d = nc.dram_tensor("router_weights", [N_LOC, TOP_K], F32, kind="ExternalOutput")
    idx_d = nc.dram_tensor("sel_idx", [N_LOC, TOP_K], I32, kind="ExternalOutput")
    mask_d = nc.dram_tensor("expert_mask", [E, TOP_K, N_LOC], I32, kind="ExternalOutput")

    mmdt = F32R if mm_f32r else F32

    with tile.TileContext(nc) as tc:
        with (
            tc.tile_pool(name="const", bufs=1) as constp,
            tc.tile_pool(name="xin", bufs=NG * NQ) as xinp,
            tc.tile_pool(name="sb", bufs=2) as sbp,
            tc.tile_pool(name="psB", bufs=2, space="PSUM") as psB,
            tc.tile_pool(name="psC", bufs=2, space="PSUM") as psC,
        ):
            # ---- small W/b loads go first on the sync ring so the PE
            # preamble (W transposes) can start immediately; bulk x loads
            # follow on the same ring; output stores use the scalar ring.
            b_sb = constp.tile([128, 1], F32)
            nc.sync.dma_start(out=b_sb[0:E, :], in_=b_d[:].rearrange("(e o) -> e o", o=1))
            nc.sync.dma_start(out=b_sb[E:2 * E, :], in_=b_d[:].rearrange("(e o) -> e o", o=1))
            w_nat = constp.tile([E, H], F32)
            nc.sync.dma_start(out=w_nat, in_=w_d[:, :])

            # ---- x loads: [128, KH, GROUP] quarter-slabs, 8KB contiguous
            # per partition
            def load_quarter(g, h):
                xh = xinp.tile([128, KH, GROUP], F32, tag="xgh",
                               name=f"xgh_{g}_{h}")
                nc.sync.dma_start(out=xh, in_=xg_d[g, :, h * KH:(h + 1) * KH, :])
                return xh

            xhalves = {}
            for g in range(NG):
                for h in range(NQ):
                    xhalves[(g, h)] = load_quarter(g, h)

            # ---- constants ----
            ident = constp.tile([128, 128], F32)
            make_identity(nc, ident)
            iota_u = constp.tile([128, E], U32)
            nc.gpsimd.iota(iota_u, pattern=[[1, E]], base=0, channel_multiplier=0)
            iota_f = constp.tile([128, E], F32)
            nc.vector.tensor_copy(out=iota_f, in_=iota_u)

            # accumulators for the packed weights/indices outputs
            w1c = constp.tile([128, QROWS], F32)
            w2c = constp.tile([128, QROWS], F32)
            i1c = constp.tile([128, QROWS], F32)
            i2c = constp.tile([128, QROWS], F32)
            comb_w = constp.tile([QROWS, 128 * TOP_K], F32)
            comb_i = constp.tile([QROWS, 128 * TOP_K], I32)

            # ---- W -> WT (h on partitions) ----
            wt = constp.tile([128, HK * E], F32)
            for k in range(HK):
                pw = psC.tile([128, E], F32, tag="plg")
                nc.tensor.transpose(pw, w_nat[:, k * 128:(k + 1) * 128], ident[:E, :E])
                nc.vector.tensor_copy(out=wt[:, k * E:(k + 1) * E], in_=pw)

            # ---- main loop over token groups ----
            HG = GROUP // 2
            for g in range(NG):
                # gate matmul: logitsT[e, tok] += WT_k.T @ xT_k.
                # Column-tiled over tokens: PE column-group 0 (stationary in
                # array cols 0-63) streams tokens 0-255 and drains to PSUM
                # partitions 0-63; column-group 1 streams tokens 256-511 and
                # drains to partitions 64-127. Both halves run concurrently.
                pltA = psB.tile([128, HG], F32, tag="pa", name=f"pltA_{g}")
                pltB = psB.tile([128, HG], F32, tag="pb", name=f"pltB_{g}")
                for k in range(HK):
                    xh = xhalves[(g, k // KH)]
                    xk = xh.bitcast(mmdt)[:, k % KH, :]
                    nc.tensor.matmul(
                        pltA[0:E, :],
                        lhsT=wt.bitcast(mmdt)[:, k * E:(k + 1) * E],
                        rhs=xk[:, 0:HG],
                        start=(k == 0),
                        stop=(k == HK - 1),
                        tile_position=(0, 0),
                    )
                    nc.tensor.matmul(
                        pltB[E:2 * E, :],
                        lhsT=wt.bitcast(mmdt)[:, k * E:(k + 1) * E],
                        rhs=xk[:, HG:GROUP],
                        start=(k == 0),
                        stop=(k == HK - 1),
                        tile_position=(0, E),
                    )

                # bias add + move to SBUF (ACT).  lt_sb partitions 0-63 hold
                # logitsT for tokens 0-255, partitions 64-127 tokens 256-511.
                lt_sb = sbp.tile([128, HG], F32, tag="lt")
                nc.scalar.add(out=lt_sb[0:E, :], in_=pltA[0:E, :], add=b_sb[0:E, :])
                nc.scalar.add(out=lt_sb[E:2 * E, :], in_=pltB[E:2 * E, :], add=b_sb[E:2 * E, :])

                # transpose logits back to [tok, e]
                plg = psC.tile([128, NT * E], F32, tag="plg")
                for t in range(NT):
                    if t < NT // 2:
                        src = lt_sb[0:E, t * 128:(t + 1) * 128]
                        idn = ident[:E, :E]
                    else:
                        tt = t - NT // 2
                        src = lt_sb[E:2 * E, tt * 128:(tt + 1) * 128]
                        idn = ident[E:2 * E, E:2 * E]
                    nc.tensor.transpose(plg[:, t * E:(t + 1) * E], src, idn)
                lg_sb = sbp.tile([128, NT * E], F32, tag="lg")
                nc.scalar.copy(out=lg_sb, in_=plg)
                nc.scalar.dma_start(
                    out=logits_d[g * GROUP:(g + 1) * GROUP, :].rearrange(
                        "(t p) e -> p t e", p=128
                    ),
                    in_=lg_sb.rearrange("p (t e) -> p t e", e=E),
                )

                # top-2 (batched over the 4 sub-tiles)
                v8 = sbp.tile([128, NT, 8], F32, tag="v8", bufs=3)
                i8 = sbp.tile([128, NT, 8], U32, tag="i8", bufs=3)
                for t in range(NT):
                    lgt = lg_sb[:, t * E:(t + 1) * E]
                    nc.vector.max(v8[:, t], lgt)
                    nc.vector.max_index(i8[:, t], v8[:, t], lgt)

                gs = g * NT
                d = sbp.tile([128, NT], F32, tag="d", bufs=3)
                nc.vector.tensor_tensor(
                    out=d, in0=v8[:, :, 1], in1=v8[:, :, 0], op=mybir.AluOpType.subtract
                )
                e2 = sbp.tile([128, NT], F32, tag="e2", bufs=3)
                nc.scalar.activation(e2, d, mybir.ActivationFunctionType.Exp)
                den = sbp.tile([128, NT], F32, tag="den", bufs=3)
                nc.vector.tensor_scalar_add(out=den, in0=e2, scalar1=1.0)
                nc.vector.reciprocal(out=w1c[:, gs:gs + NT], in_=den)
                nc.vector.tensor_tensor(
                    out=w2c[:, gs:gs + NT], in0=e2, in1=w1c[:, gs:gs + NT],
                    op=mybir.AluOpType.mult,
                )
                nc.vector.tensor_copy(out=i1c[:, gs:gs + NT], in_=i8[:, :, 0])
                nc.vector.tensor_copy(out=i2c[:, gs:gs + NT], in_=i8[:, :, 1])

                # one-hot masks -> [e, tok] via PE transpose, per 512-token
                # subgroup (keeps each PSUM mask tile within one bank)
                for s in range(SUB):
                    pm0 = psC.tile([E, 512], F32, tag="pm", name=f"pm0_{g}_{s}")
                    pm1 = psC.tile([E, 512], F32, tag="pm", name=f"pm1_{g}_{s}")
                    for tl in range(4):
                        t = s * 4 + tl
                        for pm, ic in ((pm0, i1c), (pm1, i2c)):
                            mk = sbp.tile([128, E], F32, tag="mk", bufs=3)
                            nc.vector.tensor_scalar(
                                out=mk, in0=iota_f, scalar1=ic[:, gs + t:gs + t + 1],
                                scalar2=None, op0=mybir.AluOpType.is_equal,
                            )
                            nc.tensor.transpose(
                                pm[:, tl * 128:(tl + 1) * 128], mk, ident
                            )

                    for kk, pm in ((0, pm0), (1, pm1)):
                        msk = sbp.tile([E, 512], I32, tag="msk")
                        nc.vector.tensor_copy(out=msk, in_=pm)
                        nc.scalar.dma_start(
                            out=mask_d[:, kk,
                                       g * GROUP + s * 512:g * GROUP + (s + 1) * 512],
                            in_=msk,
                        )

            # ---- pack and store router_weights / indices ----
            for src, dst, kk in (
                (w1c, comb_w, 0),
                (w2c, comb_w, 1),
                (i1c, comb_i, 0),
                (i2c, comb_i, 1),
            ):
                pp = psC.tile([QROWS, 128], F32, tag="plg", name="pp")
                nc.tensor.transpose(pp, src, ident)
                nc.vector.tensor_copy(
                    out=dst.rearrange("q (p k) -> q p k", k=TOP_K)[:, :, kk], in_=pp
                )
            nc.scalar.dma_start(
                out=rw_d[:, :].rearrange("(q p) k -> q (p k)", q=QROWS), in_=comb_w
            )
            nc.scalar.dma_start(
                out=idx_d[:, :].rearrange("(q p) k -> q (p k)", q=QROWS), in_=comb_i
            )

    nc.compile()
    return nc


_NC_CACHE = {}


def get_nc(mm_f32r=MM_F32R, tr_f32r=TR_F32R):
    key = (mm_f32r, tr_f32r)
    if key not in _NC_CACHE:
        _NC_CACHE[key] = build_nc(*key)
    return _NC_CACHE[key]


def shard_x(x):
    """Per-core [NG, 128, HK, GROUP] slabs: slab[g, p, k, t] = x[g*512+t, k*128+p]."""
    out = []
    for c in range(NCORES):
        xs = np.asarray(x[c * N_LOC:(c + 1) * N_LOC], dtype=np.float32)
        slab = np.ascontiguousarray(
            xs.T.reshape(HK, 128, NG, GROUP).transpose(2, 1, 0, 3)
        )
        out.append(slab)
    return out


def make_in_maps(x, W, b):
    W = np.ascontiguousarray(np.asarray(W, dtype=np.float32))
    b = np.ascontiguousarray(np.asarray(b, dtype=np.float32))
    return [{"xg": xs, "W": W, "b": b} for xs in shard_x(x)]


def assemble(results):
    logits = np.concatenate([r["router_logits"] for r in results], axis=0)
    weights = np.concatenate([r["router_weights"] for r in results], axis=0)
    idx = np.concatenate([r["sel_idx"] for r in results], axis=0).astype(np.int32)
    mask = np.concatenate([r["expert_mask"] for r in results], axis=2).astype(np.int32)
    return (logits, weights, idx, mask)


def kernel(x, W, b):
    from concourse.bass_utils import run_bass_kernel_spmd

    nc = get_nc()
    res = run_bass_kernel_spmd(nc, make_in_maps(x, W, b), list(range(NCORES)))
    return assemble(res.results)


# revision 31
# speedup vs baseline: 1.0213x; 1.0213x over previous
"""MoE router kernel for Trainium2 (8 NeuronCores, data-parallel over tokens).

Computes, for x (N=16384, H=2048) f32, W (E=64, H) f32, b (E,) f32:
  router_logits  (N, E) f32   = x @ W.T + b
  router_weights (N, 2) f32   = renormalized top-2 softmax probs
  selected_idx   (N, 2) int32 = top-2 expert indices
  expert_mask    (E, 2, N) int32 = one-hot(selected_idx) transposed (2,1,0)

Sharding: token dim N split evenly across 8 cores; W/b replicated. Each
core's x shard is pre-arranged on the host into per-group slabs
[NG, 128, HK, GROUP] (hidden chunk on the partition axis) so the gate
matmul contracts over H directly from DMA-friendly 32KB-contiguous rows.
"""

import os
import sys

import numpy as np

if not os.path.isdir(os.path.join(os.path.dirname(os.path.abspath(__file__)), "concourse")):
    for _p in ("/opt/trn_rl_repo",):
        if os.path.isdir(_p) and _p not in sys.path:
            sys.path.insert(0, _p)

import concourse.bass as bass  # noqa: E402,F401
import concourse.bacc as bacc  # noqa: E402
import concourse.mybir as mybir  # noqa: E402
import concourse.tile as tile  # noqa: E402

F32 = mybir.dt.float32
F32R = mybir.dt.float32r
I32 = mybir.dt.int32
U32 = mybir.dt.uint32

N = 16384
H = 2048
E = 64
TOP_K = 2
NCORES = 8
N_LOC = N // NCORES      # 2048 tokens per core
GROUP = 1024             # tokens per compute group
NG = N_LOC // GROUP      # 2 groups
NT = GROUP // 128        # 8 token sub-tiles per group
SUB = GROUP // 512       # 512-token subgroups (mask granularity)
HK = H // 128            # 16 contraction chunks
KH = HK // 4             # k-chunks per DMA quarter-slab
NQ = HK // KH            # DMA slabs per group
QROWS = N_LOC // 128     # 16 (for weights/indices packing)

MM_F32R = False
TR_F32R = False


def build_nc(mm_f32r=MM_F32R, tr_f32r=TR_F32R):
    nc = bacc.Bacc("TRN2", target_bir_lowering=False)

    xg_d = nc.dram_tensor("xg", [NG, 128, HK, GROUP], F32, kind="ExternalInput")
    w_d = nc.dram_tensor("W", [E, H], F32, kind="ExternalInput")
    b_d = nc.dram_tensor("b", [E], F32, kind="ExternalInput")
    ident_d = nc.dram_tensor("ident128", [128, 128], F32, kind="ExternalInput")
    iota_d = nc.dram_tensor("iota64", [128, E], F32, kind="ExternalInput")
    logits_d = nc.dram_tensor("router_logits", [N_LOC, E], F32, kind="ExternalOutput")
    rw_d = nc.dram_tensor("router_weights", [N_LOC, TOP_K], F32, kind="ExternalOutput")
    idx_d = nc.dram_tensor("sel_idx", [N_LOC, TOP_K], I32, kind="ExternalOutput")
    mask_d = nc.dram_tensor("expert_mask", [E, TOP_K, N_LOC], I32, kind="ExternalOutput")

    mmdt = F32R if mm_f32r else F32

    with tile.TileContext(nc) as tc:
        with (
            tc.tile_pool(name="const", bufs=1) as constp,
            tc.tile_pool(name="xin", bufs=NG * NQ) as xinp,
            tc.tile_pool(name="sb", bufs=2) as sbp,
            tc.tile_pool(name="psB", bufs=2, space="PSUM") as psB,
            tc.tile_pool(name="psC", bufs=2, space="PSUM") as psC,
        ):
            # ---- small W/b loads go first on the sync ring so the PE
            # preamble (W transposes) can start immediately; bulk x loads
            # follow on the same ring; output stores use the scalar ring.
            ident = constp.tile([128, 128], F32)
            nc.sync.dma_start(out=ident, in_=ident_d[:, :])
            iota_f = constp.tile([128, E], F32)
            nc.sync.dma_start(out=iota_f, in_=iota_d[:, :])
            b_sb = constp.tile([128, 1], F32)
            nc.sync.dma_start(out=b_sb[0:E, :], in_=b_d[:].rearrange("(e o) -> e o", o=1))
            nc.sync.dma_start(out=b_sb[E:2 * E, :], in_=b_d[:].rearrange("(e o) -> e o", o=1))
            w_nat = constp.tile([E, H], F32)
            nc.sync.dma_start(out=w_nat, in_=w_d[:, :])

            # ---- x loads: [128, KH, GROUP] quarter-slabs, 8KB contiguous
            # per partition
            def load_quarter(g, h):
                xh = xinp.tile([128, KH, GROUP], F32, tag="xgh",
                               name=f"xgh_{g}_{h}")
                nc.sync.dma_start(out=xh, in_=xg_d[g, :, h * KH:(h + 1) * KH, :])
                return xh

            xhalves = {}
            for g in range(NG):
                for h in range(NQ):
                    xhalves[(g, h)] = load_quarter(g, h)

            # accumulators for the packed weights/indices outputs
            w1c = constp.tile([128, QROWS], F32)
            w2c = constp.tile([128, QROWS], F32)
            i1c = constp.tile([128, QROWS], F32)
            i2c = constp.tile([128, QROWS], F32)
            comb_w = constp.tile([QROWS, 128 * TOP_K], F32)
            comb_i = constp.tile([QROWS, 128 * TOP_K], I32)

            # ---- W -> WT (h on partitions) ----
            wt = constp.tile([128, HK * E], F32)
            for k in range(HK):
                pw = psC.tile([128, E], F32, tag="plg")
                nc.tensor.transpose(pw, w_nat[:, k * 128:(k + 1) * 128], ident[:E, :E])
                nc.vector.tensor_copy(out=wt[:, k * E:(k + 1) * E], in_=pw)

            # ---- main loop over token groups ----
            HG = GROUP // 2
            for g in range(NG):
                # gate matmul: logitsT[e, tok] += WT_k.T @ xT_k.
                # Column-tiled over tokens: PE column-group 0 (stationary in
                # array cols 0-63) streams tokens 0-255 and drains to PSUM
                # partitions 0-63; column-group 1 streams tokens 256-511 and
                # drains to partitions 64-127. Both halves run concurrently.
                pltA = psB.tile([128, HG], F32, tag="pa", name=f"pltA_{g}")
                pltB = psB.tile([128, HG], F32, tag="pb", name=f"pltB_{g}")
                for k in range(HK):
                    xh = xhalves[(g, k // KH)]
                    xk = xh.bitcast(mmdt)[:, k % KH, :]
                    nc.tensor.matmul(
                        pltA[0:E, :],
                        lhsT=wt.bitcast(mmdt)[:, k * E:(k + 1) * E],
                        rhs=xk[:, 0:HG],
                        start=(k == 0),
                        stop=(k == HK - 1),
                        tile_position=(0, 0),
                    )
                    nc.tensor.matmul(
                        pltB[E:2 * E, :],
                        lhsT=wt.bitcast(mmdt)[:, k * E:(k + 1) * E],
                        rhs=xk[:, HG:GROUP],
                        start=(k == 0),
                        stop=(k == HK - 1),
                        tile_position=(0, E),
                    )

                # bias add + move to SBUF (ACT).  lt_sb partitions 0-63 hold
                # logitsT for tokens 0-255, partitions 64-127 tokens 256-511.
                lt_sb = sbp.tile([128, HG], F32, tag="lt")
                nc.scalar.add(out=lt_sb[0:E, :], in_=pltA[0:E, :], add=b_sb[0:E, :])
                nc.scalar.add(out=lt_sb[E:2 * E, :], in_=pltB[E:2 * E, :], add=b_sb[E:2 * E, :])

                # transpose logits back to [tok, e]
                plg = psC.tile([128, NT * E], F32, tag="plg")
                for t in range(NT):
                    if t < NT // 2:
                        src = lt_sb[0:E, t * 128:(t + 1) * 128]
                        idn = ident[:E, :E]
                    else:
                        tt = t - NT // 2
                        src = lt_sb[E:2 * E, tt * 128:(tt + 1) * 128]
                        idn = ident[E:2 * E, E:2 * E]
                    nc.tensor.transpose(plg[:, t * E:(t + 1) * E], src, idn)
                lg_sb = sbp.tile([128, NT * E], F32, tag="lg")
                nc.scalar.copy(out=lg_sb, in_=plg)
                nc.scalar.dma_start(
                    out=logits_d[g * GROUP:(g + 1) * GROUP, :].rearrange(
                        "(t p) e -> p t e", p=128
                    ),
                    in_=lg_sb.rearrange("p (t e) -> p t e", e=E),
                )

                # top-2 (batched over the 4 sub-tiles)
                v8 = sbp.tile([128, NT, 8], F32, tag="v8", bufs=3)
                i8 = sbp.tile([128, NT, 8], U32, tag="i8", bufs=3)
                for t in range(NT):
                    lgt = lg_sb[:, t * E:(t + 1) * E]
                    nc.vector.max(v8[:, t], lgt)
                    nc.vector.max_index(i8[:, t], v8[:, t], lgt)

                gs = g * NT
                d = sbp.tile([128, NT], F32, tag="d", bufs=3)
                nc.vector.tensor_tensor(
                    out=d, in0=v8[:, :, 1], in1=v8[:, :, 0], op=mybir.AluOpType.subtract
                )
                e2 = sbp.tile([128, NT], F32, tag="e2", bufs=3)
                nc.scalar.activation(e2, d, mybir.ActivationFunctionType.Exp)
                den = sbp.tile([128, NT], F32, tag="den", bufs=3)
                nc.vector.tensor_scalar_add(out=den, in0=e2, scalar1=1.0)
                nc.vector.reciprocal(out=w1c[:, gs:gs + NT], in_=den)
                nc.vector.tensor_tensor(
                    out=w2c[:, gs:gs + NT], in0=e2, in1=w1c[:, gs:gs + NT],
                    op=mybir.AluOpType.mult,
                )
                nc.vector.tensor_copy(out=i1c[:, gs:gs + NT], in_=i8[:, :, 0])
                nc.vector.tensor_copy(out=i2c[:, gs:gs + NT], in_=i8[:, :, 1])

                # one-hot masks -> [e, tok] via PE transpose, per 512-token
                # subgroup (keeps each PSUM mask tile within one bank)
                for s in range(SUB):
                    pm0 = psC.tile([E, 512], F32, tag="pm", name=f"pm0_{g}_{s}")
                    pm1 = psC.tile([E, 512], F32, tag="pm", name=f"pm1_{g}_{s}")
                    for tl in range(4):
                        t = s * 4 + tl
                        for pm, ic in ((pm0, i1c), (pm1, i2c)):
                            mk = sbp.tile([128, E], F32, tag="mk", bufs=3)
                            nc.vector.tensor_scalar(
                                out=mk, in0=iota_f, scalar1=ic[:, gs + t:gs + t + 1],
                                scalar2=None, op0=mybir.AluOpType.is_equal,
                            )
                            nc.tensor.transpose(
                                pm[:, tl * 128:(tl + 1) * 128], mk, ident
                            )

                    for kk, pm in ((0, pm0), (1, pm1)):
                        msk = sbp.tile([E, 512], I32, tag="msk")
                        nc.vector.tensor_copy(out=msk, in_=pm)
                        nc.scalar.dma_start(
                            out=mask_d[:, kk,
                                       g * GROUP + s * 512:g * GROUP + (s + 1) * 512],
                            in_=msk,
                        )

            # ---- pack and store router_weights / indices ----
            for src, dst, kk in (
                (w1c, comb_w, 0),
                (w2c, comb_w, 1),
                (i1c, comb_i, 0),
                (i2c, comb_i, 1),
            ):
                pp = psC.tile([QROWS, 128], F32, tag="plg", name="pp")
                nc.tensor.transpose(pp, src, ident)
                nc.vector.tensor_copy(
                    out=dst.rearrange("q (p k) -> q p k", k=TOP_K)[:, :, kk], in_=pp
                )
            nc.scalar.dma_start(
                out=rw_d[:, :].rearrange("(q p) k -> q (p k)", q=QROWS), in_=comb_w
            )
            nc.scalar.dma_start(
                out=idx_d[:, :].rearrange("(q p) k -> q (p k)", q=QROWS), in_=comb_i
            )

    nc.compile()
    return nc


_NC_CACHE = {}


def get_nc(mm_f32r=MM_F32R, tr_f32r=TR_F32R):
    key = (mm_f32r, tr_f32r)
    if key not in _NC_CACHE:
        _NC_CACHE[key] = build_nc(*key)
    return _NC_CACHE[key]


def shard_x(x):
    """Per-core [NG, 128, HK, GROUP] slabs: slab[g, p, k, t] = x[g*512+t, k*128+p]."""
    out = []
    for c in range(NCORES):
        xs = np.asarray(x[c * N_LOC:(c + 1) * N_LOC], dtype=np.float32)
        slab = np.ascontiguousarray(
            xs.T.reshape(HK, 128, NG, GROUP).transpose(2, 1, 0, 3)
        )
        out.append(slab)
    return out


def make_in_maps(x, W, b):
    W = np.ascontiguousarray(np.asarray(W, dtype=np.float32))
    b = np.ascontiguousarray(np.asarray(b, dtype=np.float32))
    ident = np.eye(128, dtype=np.float32)
    iota = np.broadcast_to(np.arange(E, dtype=np.float32), (128, E)).copy()
    return [
        {"xg": xs, "W": W, "b": b, "ident128": ident, "iota64": iota}
        for xs in shard_x(x)
    ]


def assemble(results):
    logits = np.concatenate([r["router_logits"] for r in results], axis=0)
    weights = np.concatenate([r["router_weights"] for r in results], axis=0)
    idx = np.concatenate([r["sel_idx"] for r in results], axis=0).astype(np.int32)
    mask = np.concatenate([r["expert_mask"] for r in results], axis=2).astype(np.int32)
    return (logits, weights, idx, mask)


def kernel(x, W, b):
    from concourse.bass_utils import run_bass_kernel_spmd

    nc = get_nc()
    res = run_bass_kernel_spmd(nc, make_in_maps(x, W, b), list(range(NCORES)))
    return assemble(res.results)


# revision 32
# speedup vs baseline: 1.0440x; 1.0223x over previous
"""MoE router kernel for Trainium2 (8 NeuronCores, data-parallel over tokens).

Computes, for x (N=16384, H=2048) f32, W (E=64, H) f32, b (E,) f32:
  router_logits  (N, E) f32   = x @ W.T + b
  router_weights (N, 2) f32   = renormalized top-2 softmax probs
  selected_idx   (N, 2) int32 = top-2 expert indices
  expert_mask    (E, 2, N) int32 = one-hot(selected_idx) transposed (2,1,0)

Sharding: token dim N split evenly across 8 cores; W/b replicated. Each
core's x shard is pre-arranged on the host into per-group slabs
[NG, 128, HK, GROUP] (hidden chunk on the partition axis) so the gate
matmul contracts over H directly from DMA-friendly 32KB-contiguous rows.
"""

import os
import sys

import numpy as np

if not os.path.isdir(os.path.join(os.path.dirname(os.path.abspath(__file__)), "concourse")):
    for _p in ("/opt/trn_rl_repo",):
        if os.path.isdir(_p) and _p not in sys.path:
            sys.path.insert(0, _p)

import concourse.bass as bass  # noqa: E402,F401
import concourse.bacc as bacc  # noqa: E402
import concourse.mybir as mybir  # noqa: E402
import concourse.tile as tile  # noqa: E402
from concourse.masks import make_identity  # noqa: E402

F32 = mybir.dt.float32
F32R = mybir.dt.float32r
I32 = mybir.dt.int32
U32 = mybir.dt.uint32

N = 16384
H = 2048
E = 64
TOP_K = 2
NCORES = 8
N_LOC = N // NCORES      # 2048 tokens per core
GROUP = 512              # tokens per compute group
NG = N_LOC // GROUP      # 4 groups
NT = GROUP // 128        # 4 token sub-tiles per group
HK = H // 128            # 16 contraction chunks
KH = HK // 4             # k-chunks per DMA quarter-slab
NQ = HK // KH            # DMA slabs per group
QROWS = N_LOC // 128     # 16 (for weights/indices packing)

MM_F32R = False
TR_F32R = False


def build_nc(mm_f32r=MM_F32R, tr_f32r=TR_F32R):
    nc = bacc.Bacc("TRN2", target_bir_lowering=False)

    xg_d = nc.dram_tensor("xg", [NG, 128, HK, GROUP], F32, kind="ExternalInput")
    w_d = nc.dram_tensor("W", [E, H], F32, kind="ExternalInput")
    b_d = nc.dram_tensor("b", [E], F32, kind="ExternalInput")
    logits_d = nc.dram_tensor("router_logits", [N_LOC, E], F32, kind="ExternalOutput")
    rw_d = nc.dram_tensor("router_weights", [N_LOC, TOP_K], F32, kind="ExternalOutput")
    idx_d = nc.dram_tensor("sel_idx", [N_LOC, TOP_K], I32, kind="ExternalOutput")
    mask_d = nc.dram_tensor("expert_mask", [E, TOP_K, N_LOC], I32, kind="ExternalOutput")

    mmdt = F32R if mm_f32r else F32

    with tile.TileContext(nc) as tc:
        with (
            tc.tile_pool(name="const", bufs=1) as constp,
            tc.tile_pool(name="xin", bufs=NG * NQ) as xinp,
            tc.tile_pool(name="sb", bufs=2) as sbp,
            tc.tile_pool(name="psB", bufs=2, space="PSUM") as psB,
            tc.tile_pool(name="psC", bufs=2, space="PSUM") as psC,
        ):
            # ---- small W/b loads go first on the sync ring so the PE
            # preamble (W transposes) can start immediately; bulk x loads
            # follow on the same ring; output stores use the scalar ring.
            b_sb = constp.tile([128, 1], F32)
            nc.sync.dma_start(out=b_sb[0:E, :], in_=b_d[:].rearrange("(e o) -> e o", o=1))
            nc.sync.dma_start(out=b_sb[E:2 * E, :], in_=b_d[:].rearrange("(e o) -> e o", o=1))
            w_nat = constp.tile([E, H], F32)
            nc.sync.dma_start(out=w_nat, in_=w_d[:, :])

            # ---- x loads: [128, KH, GROUP] quarter-slabs, 8KB contiguous
            # per partition
            def load_quarter(g, h):
                xh = xinp.tile([128, KH, GROUP], F32, tag="xgh",
                               name=f"xgh_{g}_{h}")
                nc.sync.dma_start(out=xh, in_=xg_d[g, :, h * KH:(h + 1) * KH, :])
                return xh

            xhalves = {}
            for g in range(NG):
                for h in range(NQ):
                    xhalves[(g, h)] = load_quarter(g, h)

            # ---- constants ----
            ident = constp.tile([128, 128], F32)
            make_identity(nc, ident)
            iota_u = constp.tile([128, E], U32)
            nc.gpsimd.iota(iota_u, pattern=[[1, E]], base=0, channel_multiplier=0)
            iota_f = constp.tile([128, E], F32)
            nc.vector.tensor_copy(out=iota_f, in_=iota_u)

            # accumulators for the packed weights/indices outputs
            w1c = constp.tile([128, QROWS], F32)
            w2c = constp.tile([128, QROWS], F32)
            i1c = constp.tile([128, QROWS], F32)
            i2c = constp.tile([128, QROWS], F32)
            comb_w = constp.tile([QROWS, 128 * TOP_K], F32)
            comb_i = constp.tile([QROWS, 128 * TOP_K], I32)

            # ---- W -> WT (h on partitions) ----
            wt = constp.tile([128, HK * E], F32)
            for k in range(HK):
                pw = psC.tile([128, E], F32, tag="plg")
                nc.tensor.transpose(pw, w_nat[:, k * 128:(k + 1) * 128], ident[:E, :E])
                nc.vector.tensor_copy(out=wt[:, k * E:(k + 1) * E], in_=pw)

            # ---- main loop over token groups ----
            HG = GROUP // 2
            for g in range(NG):
                # gate matmul: logitsT[e, tok] += WT_k.T @ xT_k.
                # Column-tiled over tokens: PE column-group 0 (stationary in
                # array cols 0-63) streams tokens 0-255 and drains to PSUM
                # partitions 0-63; column-group 1 streams tokens 256-511 and
                # drains to partitions 64-127. Both halves run concurrently.
                pltA = psB.tile([128, HG], F32, tag="pa", name=f"pltA_{g}")
                pltB = psB.tile([128, HG], F32, tag="pb", name=f"pltB_{g}")
                for k in range(HK):
                    xh = xhalves[(g, k // KH)]
                    xk = xh.bitcast(mmdt)[:, k % KH, :]
                    nc.tensor.matmul(
                        pltA[0:E, :],
                        lhsT=wt.bitcast(mmdt)[:, k * E:(k + 1) * E],
                        rhs=xk[:, 0:HG],
                        start=(k == 0),
                        stop=(k == HK - 1),
                        tile_position=(0, 0),
                    )
                    nc.tensor.matmul(
                        pltB[E:2 * E, :],
                        lhsT=wt.bitcast(mmdt)[:, k * E:(k + 1) * E],
                        rhs=xk[:, HG:GROUP],
                        start=(k == 0),
                        stop=(k == HK - 1),
                        tile_position=(0, E),
                    )

                # bias add + move to SBUF (ACT).  lt_sb partitions 0-63 hold
                # logitsT for tokens 0-255, partitions 64-127 tokens 256-511.
                lt_sb = sbp.tile([128, HG], F32, tag="lt")
                nc.scalar.add(out=lt_sb[0:E, :], in_=pltA[0:E, :], add=b_sb[0:E, :])
                nc.scalar.add(out=lt_sb[E:2 * E, :], in_=pltB[E:2 * E, :], add=b_sb[E:2 * E, :])

                # transpose logits back to [tok, e]
                plg = psC.tile([128, NT * E], F32, tag="plg")
                for t in range(NT):
                    if t < 2:
                        src = lt_sb[0:E, t * 128:(t + 1) * 128]
                        idn = ident[:E, :E]
                    else:
                        src = lt_sb[E:2 * E, (t - 2) * 128:(t - 1) * 128]
                        idn = ident[E:2 * E, E:2 * E]
                    nc.tensor.transpose(plg[:, t * E:(t + 1) * E], src, idn)
                lg_sb = sbp.tile([128, NT * E], F32, tag="lg")
                nc.scalar.copy(out=lg_sb, in_=plg)
                nc.scalar.dma_start(
                    out=logits_d[g * GROUP:(g + 1) * GROUP, :].rearrange(
                        "(t p) e -> p t e", p=128
                    ),
                    in_=lg_sb.rearrange("p (t e) -> p t e", e=E),
                )

                # top-2 (batched over the 4 sub-tiles)
                v8 = sbp.tile([128, NT, 8], F32, tag="v8", bufs=3)
                i8 = sbp.tile([128, NT, 8], U32, tag="i8", bufs=3)
                for t in range(NT):
                    lgt = lg_sb[:, t * E:(t + 1) * E]
                    nc.vector.max(v8[:, t], lgt)
                    nc.vector.max_index(i8[:, t], v8[:, t], lgt)

                gs = g * NT
                d = sbp.tile([128, NT], F32, tag="d", bufs=3)
                nc.vector.tensor_tensor(
                    out=d, in0=v8[:, :, 1], in1=v8[:, :, 0], op=mybir.AluOpType.subtract
                )
                e2 = sbp.tile([128, NT], F32, tag="e2", bufs=3)
                nc.scalar.activation(e2, d, mybir.ActivationFunctionType.Exp)
                den = sbp.tile([128, NT], F32, tag="den", bufs=3)
                nc.vector.tensor_scalar_add(out=den, in0=e2, scalar1=1.0)
                nc.vector.reciprocal(out=w1c[:, gs:gs + NT], in_=den)
                nc.vector.tensor_tensor(
                    out=w2c[:, gs:gs + NT], in0=e2, in1=w1c[:, gs:gs + NT],
                    op=mybir.AluOpType.mult,
                )
                nc.vector.tensor_copy(out=i1c[:, gs:gs + NT], in_=i8[:, :, 0])
                nc.vector.tensor_copy(out=i2c[:, gs:gs + NT], in_=i8[:, :, 1])

                # one-hot masks -> [e, tok] via PE transpose
                pm0 = psC.tile([E, GROUP], F32, tag="pm", name="pm0")
                pm1 = psC.tile([E, GROUP], F32, tag="pm", name="pm1")
                for t in range(NT):
                    for pm, ic in ((pm0, i1c), (pm1, i2c)):
                        mk = sbp.tile([128, E], F32, tag="mk", bufs=3)
                        nc.vector.tensor_scalar(
                            out=mk, in0=iota_f, scalar1=ic[:, gs + t:gs + t + 1],
                            scalar2=None, op0=mybir.AluOpType.is_equal,
                        )
                        nc.tensor.transpose(
                            pm[:, t * 128:(t + 1) * 128], mk, ident
                        )

                for kk, pm in ((0, pm0), (1, pm1)):
                    msk = sbp.tile([E, GROUP], I32, tag="msk")
                    nc.vector.tensor_copy(out=msk, in_=pm)
                    nc.scalar.dma_start(
                        out=mask_d[:, kk, g * GROUP:(g + 1) * GROUP], in_=msk
                    )

            # ---- pack and store router_weights / indices ----
            for src, dst, kk in (
                (w1c, comb_w, 0),
                (w2c, comb_w, 1),
                (i1c, comb_i, 0),
                (i2c, comb_i, 1),
            ):
                pp = psC.tile([QROWS, 128], F32, tag="plg", name="pp")
                nc.tensor.transpose(pp, src, ident)
                nc.vector.tensor_copy(
                    out=dst.rearrange("q (p k) -> q p k", k=TOP_K)[:, :, kk], in_=pp
                )
            nc.scalar.dma_start(
                out=rw_d[:, :].rearrange("(q p) k -> q (p k)", q=QROWS), in_=comb_w
            )
            nc.scalar.dma_start(
                out=idx_d[:, :].rearrange("(q p) k -> q (p k)", q=QROWS), in_=comb_i
            )

    nc.compile()
    return nc


_NC_CACHE = {}


def get_nc(mm_f32r=MM_F32R, tr_f32r=TR_F32R):
    key = (mm_f32r, tr_f32r)
    if key not in _NC_CACHE:
        _NC_CACHE[key] = build_nc(*key)
    return _NC_CACHE[key]


def shard_x(x):
    """Per-core [NG, 128, HK, GROUP] slabs: slab[g, p, k, t] = x[g*512+t, k*128+p]."""
    out = []
    for c in range(NCORES):
        xs = np.asarray(x[c * N_LOC:(c + 1) * N_LOC], dtype=np.float32)
        slab = np.ascontiguousarray(
            xs.T.reshape(HK, 128, NG, GROUP).transpose(2, 1, 0, 3)
        )
        out.append(slab)
    return out


def make_in_maps(x, W, b):
    W = np.ascontiguousarray(np.asarray(W, dtype=np.float32))
    b = np.ascontiguousarray(np.asarray(b, dtype=np.float32))
    return [{"xg": xs, "W": W, "b": b} for xs in shard_x(x)]


def assemble(results):
    logits = np.concatenate([r["router_logits"] for r in results], axis=0)
    weights = np.concatenate([r["router_weights"] for r in results], axis=0)
    idx = np.concatenate([r["sel_idx"] for r in results], axis=0).astype(np.int32)
    mask = np.concatenate([r["expert_mask"] for r in results], axis=2).astype(np.int32)
    return (logits, weights, idx, mask)


def kernel(x, W, b):
    from concourse.bass_utils import run_bass_kernel_spmd

    nc = get_nc()
    res = run_bass_kernel_spmd(nc, make_in_maps(x, W, b), list(range(NCORES)))
    return assemble(res.results)


# revision 33
# speedup vs baseline: 1.0979x; 1.0516x over previous
"""MoE router kernel for Trainium2 (8 NeuronCores, data-parallel over tokens).

Computes, for x (N=16384, H=2048) f32, W (E=64, H) f32, b (E,) f32:
  router_logits  (N, E) f32   = x @ W.T + b
  router_weights (N, 2) f32   = renormalized top-2 softmax probs
  selected_idx   (N, 2) int32 = top-2 expert indices
  expert_mask    (E, 2, N) int32 = one-hot(selected_idx) transposed (2,1,0)

Sharding: token dim N split evenly across 8 cores; W/b replicated. Each
core's x shard is pre-arranged on the host into per-group slabs
[NG, 128, HK, GROUP] (hidden chunk on the partition axis) so the gate
matmul contracts over H directly from DMA-friendly 32KB-contiguous rows.
"""

import os
import sys

import numpy as np

if not os.path.isdir(os.path.join(os.path.dirname(os.path.abspath(__file__)), "concourse")):
    for _p in ("/opt/trn_rl_repo",):
        if os.path.isdir(_p) and _p not in sys.path:
            sys.path.insert(0, _p)

import concourse.bass as bass  # noqa: E402,F401
import concourse.bacc as bacc  # noqa: E402
import concourse.mybir as mybir  # noqa: E402
import concourse.tile as tile  # noqa: E402
from concourse.masks import make_identity  # noqa: E402

F32 = mybir.dt.float32
F32R = mybir.dt.float32r
I32 = mybir.dt.int32
U32 = mybir.dt.uint32

N = 16384
H = 2048
E = 64
TOP_K = 2
NCORES = 8
N_LOC = N // NCORES      # 2048 tokens per core
GROUP = 512              # tokens per compute group
NG = N_LOC // GROUP      # 4 groups
NT = GROUP // 128        # 4 token sub-tiles per group
HK = H // 128            # 16 contraction chunks
KH = HK // 4             # k-chunks per DMA quarter-slab
NQ = HK // KH            # DMA slabs per group
QROWS = N_LOC // 128     # 16 (for weights/indices packing)

MM_F32R = False
TR_F32R = False


def build_nc(mm_f32r=MM_F32R, tr_f32r=TR_F32R):
    nc = bacc.Bacc("TRN2", target_bir_lowering=False)

    xg_d = nc.dram_tensor("xg", [NG, 128, HK, GROUP], F32, kind="ExternalInput")
    w_d = nc.dram_tensor("W", [E, H], F32, kind="ExternalInput")
    b_d = nc.dram_tensor("b", [E], F32, kind="ExternalInput")
    logits_d = nc.dram_tensor("router_logits", [N_LOC, E], F32, kind="ExternalOutput")
    rw_d = nc.dram_tensor("router_weights", [N_LOC, TOP_K], F32, kind="ExternalOutput")
    idx_d = nc.dram_tensor("sel_idx", [N_LOC, TOP_K], I32, kind="ExternalOutput")
    mask_d = nc.dram_tensor("expert_mask", [E, TOP_K, N_LOC], I32, kind="ExternalOutput")

    mmdt = F32R if mm_f32r else F32

    with tile.TileContext(nc) as tc:
        with (
            tc.tile_pool(name="const", bufs=1) as constp,
            tc.tile_pool(name="xin", bufs=NG * NQ) as xinp,
            tc.tile_pool(name="sb", bufs=2) as sbp,
            tc.tile_pool(name="psB", bufs=2, space="PSUM") as psB,
            tc.tile_pool(name="psC", bufs=2, space="PSUM") as psC,
        ):
            # ---- small W/b loads go first on the sync ring so the PE
            # preamble (W transposes) can start immediately; bulk x loads
            # follow on the same ring; output stores use the scalar ring.
            b_sb = constp.tile([128, 1], F32)
            nc.sync.dma_start(out=b_sb[0:E, :], in_=b_d[:].rearrange("(e o) -> e o", o=1))
            nc.sync.dma_start(out=b_sb[E:2 * E, :], in_=b_d[:].rearrange("(e o) -> e o", o=1))
            w_nat = constp.tile([E, H], F32)
            nc.sync.dma_start(out=w_nat, in_=w_d[:, :])

            # ---- x loads: [128, KH, GROUP] quarter-slabs, 8KB contiguous
            # per partition
            def load_quarter(g, h):
                xh = xinp.tile([128, KH, GROUP], F32, tag="xgh",
                               name=f"xgh_{g}_{h}")
                nc.sync.dma_start(out=xh, in_=xg_d[g, :, h * KH:(h + 1) * KH, :])
                return xh

            xhalves = {}
            for g in range(NG):
                for h in range(NQ):
                    xhalves[(g, h)] = load_quarter(g, h)

            # ---- constants ----
            ident = constp.tile([128, 128], F32)
            make_identity(nc, ident)
            iota_u = constp.tile([128, E], U32)
            nc.gpsimd.iota(iota_u, pattern=[[1, E]], base=0, channel_multiplier=0)
            iota_f = constp.tile([128, E], F32)
            nc.vector.tensor_copy(out=iota_f, in_=iota_u)

            # accumulators for the packed weights/indices outputs
            w1c = constp.tile([128, QROWS], F32)
            w2c = constp.tile([128, QROWS], F32)
            i1c = constp.tile([128, QROWS], F32)
            i2c = constp.tile([128, QROWS], F32)
            comb_w = constp.tile([QROWS, 128 * TOP_K], F32)
            comb_i = constp.tile([QROWS, 128 * TOP_K], I32)

            # ---- W -> WT (h on partitions) ----
            wt = constp.tile([128, HK * E], F32)
            for k in range(HK):
                pw = psC.tile([128, E], F32, tag="plg")
                nc.tensor.transpose(pw, w_nat[:, k * 128:(k + 1) * 128], ident[:E, :E])
                nc.vector.tensor_copy(out=wt[:, k * E:(k + 1) * E], in_=pw)

            # ---- main loop over token groups ----
            HG = GROUP // 2
            for g in range(NG):
                # gate matmul: logitsT[e, tok] += WT_k.T @ xT_k.
                # Column-tiled over tokens: PE column-group 0 (stationary in
                # array cols 0-63) streams tokens 0-255 and drains to PSUM
                # partitions 0-63; column-group 1 streams tokens 256-511 and
                # drains to partitions 64-127. Both halves run concurrently.
                pltA = psB.tile([128, HG], F32, tag="pa", name=f"pltA_{g}")
                pltB = psB.tile([128, HG], F32, tag="pb", name=f"pltB_{g}")
                for k in range(HK):
                    xh = xhalves[(g, k // KH)]
                    xk = xh.bitcast(mmdt)[:, k % KH, :]
                    nc.tensor.matmul(
                        pltA[0:E, :],
                        lhsT=wt.bitcast(mmdt)[:, k * E:(k + 1) * E],
                        rhs=xk[:, 0:HG],
                        start=(k == 0),
                        stop=(k == HK - 1),
                        tile_position=(0, 0),
                    )
                    nc.tensor.matmul(
                        pltB[E:2 * E, :],
                        lhsT=wt.bitcast(mmdt)[:, k * E:(k + 1) * E],
                        rhs=xk[:, HG:GROUP],
                        start=(k == 0),
                        stop=(k == HK - 1),
                        tile_position=(0, E),
                    )

                # bias add + move to SBUF (ACT).  lt_sb partitions 0-63 hold
                # logitsT for tokens 0-255, partitions 64-127 tokens 256-511.
                lt_sb = sbp.tile([128, HG], F32, tag="lt")
                nc.scalar.add(out=lt_sb[0:E, :], in_=pltA[0:E, :], add=b_sb[0:E, :])
                nc.scalar.add(out=lt_sb[E:2 * E, :], in_=pltB[E:2 * E, :], add=b_sb[E:2 * E, :])

                # transpose logits back to [tok, e]
                plg = psC.tile([128, NT * E], F32, tag="plg")
                for t in range(NT):
                    if t < 2:
                        src = lt_sb[0:E, t * 128:(t + 1) * 128]
                        idn = ident[:E, :E]
                    else:
                        src = lt_sb[E:2 * E, (t - 2) * 128:(t - 1) * 128]
                        idn = ident[E:2 * E, E:2 * E]
                    nc.tensor.transpose(plg[:, t * E:(t + 1) * E], src, idn)
                lg_sb = sbp.tile([128, NT * E], F32, tag="lg")
                for t in range(NT):
                    nc.scalar.copy(
                        out=lg_sb[:, t * E:(t + 1) * E], in_=plg[:, t * E:(t + 1) * E]
                    )
                nc.scalar.dma_start(
                    out=logits_d[g * GROUP:(g + 1) * GROUP, :].rearrange(
                        "(t p) e -> p t e", p=128
                    ),
                    in_=lg_sb.rearrange("p (t e) -> p t e", e=E),
                )

                # top-2 per sub-tile, with the one-hot mask compares and
                # transposes interleaved right behind each max_index
                gs = g * NT
                v8 = sbp.tile([128, NT, 8], F32, tag="v8", bufs=3)
                i8 = sbp.tile([128, NT, 8], U32, tag="i8", bufs=3)
                pm0 = psC.tile([E, GROUP], F32, tag="pm", name="pm0")
                pm1 = psC.tile([E, GROUP], F32, tag="pm", name="pm1")
                for t in range(NT):
                    lgt = lg_sb[:, t * E:(t + 1) * E]
                    nc.vector.max(v8[:, t], lgt)
                    nc.vector.max_index(i8[:, t], v8[:, t], lgt)
                    nc.scalar.copy(out=i1c[:, gs + t:gs + t + 1], in_=i8[:, t, 0:1])
                    nc.scalar.copy(out=i2c[:, gs + t:gs + t + 1], in_=i8[:, t, 1:2])
                    for pm, ic in ((pm0, i1c), (pm1, i2c)):
                        mk = sbp.tile([128, E], F32, tag="mk", bufs=3)
                        nc.vector.tensor_scalar(
                            out=mk, in0=iota_f, scalar1=ic[:, gs + t:gs + t + 1],
                            scalar2=None, op0=mybir.AluOpType.is_equal,
                        )
                        nc.tensor.transpose(
                            pm[:, t * 128:(t + 1) * 128], mk, ident
                        )

                d = sbp.tile([128, NT], F32, tag="d", bufs=3)
                nc.vector.tensor_tensor(
                    out=d, in0=v8[:, :, 1], in1=v8[:, :, 0], op=mybir.AluOpType.subtract
                )
                e2 = sbp.tile([128, NT], F32, tag="e2", bufs=3)
                nc.scalar.activation(e2, d, mybir.ActivationFunctionType.Exp)
                den = sbp.tile([128, NT], F32, tag="den", bufs=3)
                nc.scalar.add(out=den, in_=e2, add=1.0)
                nc.vector.reciprocal(out=w1c[:, gs:gs + NT], in_=den)
                nc.vector.tensor_tensor(
                    out=w2c[:, gs:gs + NT], in0=e2, in1=w1c[:, gs:gs + NT],
                    op=mybir.AluOpType.mult,
                )

                for kk, pm, eng in ((0, pm0, nc.vector), (1, pm1, nc.vector)):
                    msk = sbp.tile([E, GROUP], I32, tag="msk")
                    eng.tensor_copy(out=msk, in_=pm)
                    nc.scalar.dma_start(
                        out=mask_d[:, kk, g * GROUP:(g + 1) * GROUP], in_=msk
                    )

            # ---- pack and store router_weights / indices ----
            for src, dst, kk in (
                (w1c, comb_w, 0),
                (w2c, comb_w, 1),
                (i1c, comb_i, 0),
                (i2c, comb_i, 1),
            ):
                pp = psC.tile([QROWS, 128], F32, tag="plg", name="pp")
                nc.tensor.transpose(pp, src, ident)
                nc.vector.tensor_copy(
                    out=dst.rearrange("q (p k) -> q p k", k=TOP_K)[:, :, kk], in_=pp
                )
            nc.scalar.dma_start(
                out=rw_d[:, :].rearrange("(q p) k -> q (p k)", q=QROWS), in_=comb_w
            )
            nc.scalar.dma_start(
                out=idx_d[:, :].rearrange("(q p) k -> q (p k)", q=QROWS), in_=comb_i
            )

    nc.compile()
    return nc


_NC_CACHE = {}


def get_nc(mm_f32r=MM_F32R, tr_f32r=TR_F32R):
    key = (mm_f32r, tr_f32r)
    if key not in _NC_CACHE:
        _NC_CACHE[key] = build_nc(*key)
    return _NC_CACHE[key]


def shard_x(x):
    """Per-core [NG, 128, HK, GROUP] slabs: slab[g, p, k, t] = x[g*512+t, k*128+p]."""
    out = []
    for c in range(NCORES):
        xs = np.asarray(x[c * N_LOC:(c + 1) * N_LOC], dtype=np.float32)
        slab = np.ascontiguousarray(
            xs.T.reshape(HK, 128, NG, GROUP).transpose(2, 1, 0, 3)
        )
        out.append(slab)
    return out


def make_in_maps(x, W, b):
    W = np.ascontiguousarray(np.asarray(W, dtype=np.float32))
    b = np.ascontiguousarray(np.asarray(b, dtype=np.float32))
    return [{"xg": xs, "W": W, "b": b} for xs in shard_x(x)]


def assemble(results):
    logits = np.concatenate([r["router_logits"] for r in results], axis=0)
    weights = np.concatenate([r["router_weights"] for r in results], axis=0)
    idx = np.concatenate([r["sel_idx"] for r in results], axis=0).astype(np.int32)
    mask = np.concatenate([r["expert_mask"] for r in results], axis=2).astype(np.int32)
    return (logits, weights, idx, mask)


def kernel(x, W, b):
    from concourse.bass_utils import run_bass_kernel_spmd

    nc = get_nc()
    res = run_bass_kernel_spmd(nc, make_in_maps(x, W, b), list(range(NCORES)))
    return assemble(res.results)
